# revision 1
# baseline (speedup 1.0000x reference)
"""Causal multi-head attention (B=2, T=2048, DIM=1024, H=16) on 8 TRN2 cores.

Sharding: core c handles batch b = c // 4 and head-group g = c % 4 (4 heads,
head-dim slice of 256).  Each core computes QKV projections for its heads,
causal attention, and a partial output projection y_partial = o_g @ wo[:, g].T
of shape (2048, 1024).  Host sums the 4 partials per batch (the tensor-parallel
all-reduce, done on host as the unshard step).

All matmuls run in float32r (fp32 with 11-bit mantissa, full PE rate).  Inputs
are pre-rounded to fp32r on host; end-to-end error vs the fp32 reference is
~2e-4.

Device layout (T=2048 tokens of one batch, DH=256 head dims of one group):
  xt   [DIM, T]    x transposed (contraction over DIM needs DIM on partitions)
  qT/kT[128, 2, T] per pair p of 2 heads; partitions = 2x64 head dims
  v    [128, 16, 4, 65]  [t-tile, k-in-tile, head, head-dim + ones column]
  scores sT[k, q] via matmul(lhsT=kT, rhs=qT); softmax without max-subtraction
  (scores ~N(0,1)); denominator accumulated by the ones column of v during
  attn@v; normalization applied to oT via ones-outer-product broadcast.
"""

import sys

sys.path.insert(0, "/opt/trn_rl_repo")

import numpy as np

B, T, DIM, H = 2, 2048, 1024, 16
HD = DIM // H          # 64
NCORES = 8
GROUPS = 4             # head-groups (one per core pair-of-batches)
GH = H // GROUPS       # 4 heads per group
DH = GH * HD           # 256 head dims per group
NPAIR = 2              # pairs of heads per group (2 heads = 128 partitions)
TT = T // 128          # 16 t-tiles
TG = T // 512          # 4 q-groups of 512


def _round_f32r(x: np.ndarray) -> np.ndarray:
    """Round fp32 to fp32r (11-bit mantissa, RNE) as the PE expects."""
    u = np.ascontiguousarray(x, np.float32).view(np.uint32).astype(np.uint64)
    u = (u + 0x800 + ((u >> 12) & 1)) & 0xFFFFF000
    return u.astype(np.uint32).view(np.float32)


def _build_program(loop=1):
    import concourse.bass as bass
    import concourse.tile as tile
    from concourse import bacc, mybir

    F32 = mybir.dt.float32
    F32R = mybir.dt.float32r
    AF = mybir.ActivationFunctionType

    nc = bacc.Bacc("TRN2", target_bir_lowering=False, debug=False,
                   num_devices=NCORES)

    xt_d = nc.dram_tensor("xt", [DIM, T], F32R, kind="ExternalInput")
    wqt_d = nc.dram_tensor("wqt", [DIM, DH], F32R, kind="ExternalInput")
    wkt_d = nc.dram_tensor("wkt", [DIM, DH], F32R, kind="ExternalInput")
    wvt_d = nc.dram_tensor("wvt", [DIM, DH], F32R, kind="ExternalInput")
    wot_d = nc.dram_tensor("wot", [DH, DIM], F32R, kind="ExternalInput")
    y_d = nc.dram_tensor("y", [T, DIM], F32, kind="ExternalOutput")

    KO = DIM // 128  # 8 contraction chunks

    with tile.TileContext(nc) as tc:
        with (
            tc.tile_pool(name="singles", bufs=1) as singles,
            tc.tile_pool(name="workp", bufs=4) as workp,
            tc.tile_pool(name="worky", bufs=4) as worky,
            tc.tile_pool(name="tiny", bufs=3) as tiny,
            tc.tile_pool(name="ps", bufs=3, space="PSUM") as ps,
            tc.tile_pool(name="pso", bufs=2, space="PSUM") as pso,
            tc.tile_pool(name="dramp", bufs=2, space="DRAM") as dramp,
        ):
            # ---- persistent SBUF tensors ----
            qT = singles.tile([128, NPAIR, T], F32R)
            kT = singles.tile([128, NPAIR, T], F32R)
            v = singles.tile([128, TT, GH, HD + 1], F32R)
            oT = singles.tile([128, NPAIR, T], F32R)

            mask01 = singles.tile([128, 128], F32)
            nc.gpsimd.memset(mask01[:], 1.0)
            # keep 1 where q - k >= 0 (k on partitions, q on free), else 0
            nc.gpsimd.affine_select(
                out=mask01[:], in_=mask01[:],
                compare_op=mybir.AluOpType.is_ge, fill=0.0,
                base=0, pattern=[[1, 128]], channel_multiplier=-1,
            )
            ones_f = singles.tile([128, HD], F32)
            nc.vector.memset(ones_f[:], 1.0)
            ones64 = singles.tile([1, HD], F32R)
            nc.vector.tensor_copy(ones64[:], ones_f[0:1, :])
            # ones column of v (denominator accumulator)
            for h in range(GH):
                nc.vector.tensor_copy(v[:, :, h, HD:HD + 1], ones_f[:, 0:TT, None])
            # warm the ACT exp table during the initial DMA
            dummy = singles.tile([128, 1], F32)
            nc.scalar.activation(dummy[:], ones_f[:, 0:1], AF.Exp)

            # ---- device-side repetition for timing (loop > 1) ----
            for _it in range(loop):
              # ---- phase 1: projections ----
              with (tc.tile_pool(name=f"wpool{_it}", bufs=1) as wpool,
                    tc.tile_pool(name=f"xqpool{_it}", bufs=3) as xqpool):
                  xt_r = xt_d.rearrange("(ko p) t -> p ko t", p=128)
                  wqt_sb = wpool.tile([128, KO, DH], F32R)
                  wkt_sb = wpool.tile([128, KO, DH], F32R)
                  wvt_sb = wpool.tile([128, KO, DH], F32R)
                  wot_sb = wpool.tile([128, DH // 128, DIM], F32R)
                  from concourse.bass import _add_dep_helper
                  wqt_r = wqt_d.rearrange("(ko p) d -> p ko d", p=128)
                  nc.sync.dma_start(wqt_sb[:, :, 0:128], wqt_r[:, :, 0:128])
                  nc.sync.dma_start(wqt_sb[:, :, 128:DH], wqt_r[:, :, 128:DH])
                  # xt quarters rotate through 3 slots; later loads chained so
                  # each gets full bandwidth and arrives in order
                  xq = [xqpool.tile([128, KO, 512], F32R, tag="xq",
                                    name=f"xq{_it}_{i}") for i in range(4)]
                  sub0 = []
                  for k2 in range(4):
                      sub0.append(nc.sync.dma_start(
                          xq[0][:, 2 * k2:2 * k2 + 2, :],
                          xt_r[:, 2 * k2:2 * k2 + 2, 0:512]))
                  nc.sync.dma_start(wkt_sb, wkt_d.rearrange("(ko p) d -> p ko d", p=128))
                  nc.sync.dma_start(wvt_sb, wvt_d.rearrange("(ko p) d -> p ko d", p=128))
                  prev = sub0[-1]
                  for quar in range(1, 4):
                      d = nc.sync.dma_start(
                          xq[quar], xt_r[:, :, 512 * quar:512 * (quar + 1)])
                      _add_dep_helper(d.ins, prev.ins, sync=True,
                                      reason="chain xt quarter loads")
                      prev = d
                  nc.sync.dma_start(wot_sb,
                                    wot_d.rearrange("(ko p) j -> p ko j", p=128))

                  # quarter-granular: compute for quarter i while i+1 loads
                  for quar in range(4):
                      qsl = slice(512 * quar, 512 * (quar + 1))
                      # q/k: one 2-bank psum holds both pairs of one quarter
                      for w_sb, dst in ((wqt_sb, qT), (wkt_sb, kT)):
                          acc = ps.tile([128, 1024], F32, tag="big")
                          for p in range(NPAIR):
                              for ko in range(KO):
                                  nc.tensor.matmul(
                                      acc[:, 512 * p:512 * (p + 1)],
                                      w_sb[:, ko, 128 * p:128 * (p + 1)],
                                      xq[quar][:, ko, :],
                                      start=(ko == 0), stop=(ko == KO - 1),
                                  )
                          nc.scalar.copy(
                              dst[:, :, qsl],
                              acc[:].rearrange("par (p t) -> par p t", p=NPAIR))

                      # v: [t, d] layout, psum [128(t), 256(d)]
                      for tt in range(4 * quar, 4 * (quar + 1)):
                          acc = pso.tile([128, DH], F32, tag="small")
                          for ko in range(KO):
                              nc.tensor.matmul(
                                  acc[:],
                                  xq[tt // 4][:, ko, 128 * (tt % 4):128 * (tt % 4 + 1)],
                                  wvt_sb[:, ko, :],
                                  start=(ko == 0), stop=(ko == KO - 1),
                              )
                          # single strided copy: [128, 4(h), 64], dst stride 65
                          nc.vector.tensor_copy(
                              v[:, tt, :, 0:HD],
                              acc[:].rearrange("p (h d) -> p h d", h=GH))

                  # ---- phase 2: causal attention (G-major) + delayed y ----
                  def emit_y_group(G):
                      # output projection for q-group G; the last group uses
                      # 1-bank chunks for a tighter end-of-kernel pipeline
                      if True:
                          for tt in range(4 * G, 4 * (G + 1)):
                              acc = ps.tile([128, 1024], F32, tag="big")
                              for jh in range(2):
                                  for p in range(NPAIR):
                                      nc.tensor.matmul(
                                          acc[:, 512 * jh:512 * (jh + 1)],
                                          oT[:, p, 128 * tt:128 * (tt + 1)],
                                          wot_sb[:, p, 512 * jh:512 * (jh + 1)],
                                          start=(p == 0), stop=(p == NPAIR - 1),
                                      )
                              ysb = worky.tile([128, 1024], F32, tag="ysb")
                              # drain halves on DVE and ACT in parallel
                              nc.vector.tensor_copy(ysb[:, 0:512], acc[:, 0:512])
                              nc.scalar.copy(ysb[:, 512:1024], acc[:, 512:1024])
                              nc.sync.dma_start(
                                  y_d[128 * tt:128 * (tt + 1), 0:512],
                                  ysb[:, 0:512])
                              nc.sync.dma_start(
                                  y_d[128 * tt:128 * (tt + 1), 512:1024],
                                  ysb[:, 512:1024])
                      else:
                          for tt in range(4 * G, 4 * (G + 1)):
                              for jh in range(2):
                                  acc = pso.tile([128, 512], F32, tag="small",
                                                 name=f"yc_{_it}_{tt}_{jh}")
                                  for p in range(NPAIR):
                                      nc.tensor.matmul(
                                          acc[:],
                                          oT[:, p, 128 * tt:128 * (tt + 1)],
                                          wot_sb[:, p, 512 * jh:512 * (jh + 1)],
                                          start=(p == 0), stop=(p == NPAIR - 1),
                                      )
                                  ysb = worky.tile([128, 512], F32, tag="ysc")
                                  if jh == 0:
                                      nc.vector.tensor_copy(ysb[:], acc[:])
                                  else:
                                      nc.scalar.copy(ysb[:], acc[:])
                                  nc.sync.dma_start(
                                      y_d[128 * tt:128 * (tt + 1),
                                          512 * jh:512 * (jh + 1)], ysb)

                  for p in range(NPAIR):
                      for G in range(TG):
                          hA, hB = 2 * p, 2 * p + 1
                          oA = pso.tile([HD + 1, 512], F32, tag="small",
                                        name=f"oA_{_it}_{p}_{G}")
                          oB = pso.tile([HD + 1, 512], F32, tag="small",
                                        name=f"oB_{_it}_{p}_{G}")
                          njt = 4 * G + 4  # causal: k-tiles 0 .. 4G+3
                          for j in range(njt):
                              dlt = j - 4 * G
                              off = max(0, dlt) * 128
                              qs = slice(512 * G + off, 512 * (G + 1))
                              ks = slice(128 * j, 128 * (j + 1))
                              # scores for both heads into one 2-bank psum tile
                              sAB = ps.tile([128, 1024], F32, tag="big")
                              nc.tensor.matmul(sAB[:, off:512],
                                               kT[0:64, p, ks], qT[0:64, p, qs],
                                               start=True, stop=True)
                              nc.tensor.matmul(sAB[:, 512 + off:1024],
                                               kT[64:128, p, ks],
                                               qT[64:128, p, qs],
                                               start=True, stop=True)
                              pAB = workp.tile([128, 1024], F32R, tag="pT")
                              nc.scalar.activation(pAB[:, off:], sAB[:, off:],
                                                   AF.Exp)
                              if dlt >= 0:  # diagonal: multiplicative mask
                                  dst = pAB[:].rearrange(
                                      "p (two q) -> p two q",
                                      two=2)[:, :, off:off + 128]
                                  nc.vector.tensor_mul(
                                      dst, dst,
                                      mask01[:, None, :].to_broadcast(
                                          (128, 2, 128)))
                              nc.tensor.matmul(oA[:, off:],
                                               v[:, j, hA, :], pAB[:, off:512],
                                               start=(j == 0),
                                               stop=(j == njt - 1))
                              nc.tensor.matmul(oB[:, off:],
                                               v[:, j, hB, :],
                                               pAB[:, 512 + off:1024],
                                               start=(j == 0),
                                               stop=(j == njt - 1))
                          # drain o psum to SBUF immediately; normalization is
                          # off the critical path and avoids PE and PSUM
                          for sigma, po in ((0, oA), (1, oB)):
                              oU = tiny.tile([HD + 1, 512], F32, tag="oU")
                              nc.vector.tensor_copy(oU[:], po[:])
                              r0 = tiny.tile([1, 512], F32, tag="r0")
                              nc.vector.reciprocal(r0[:], oU[HD:HD + 1, :])
                              # broadcast 1/denom to 64 rows via DRAM bounce
                              rdr = dramp.tile([1, 512], F32)
                              nc.sync.dma_start(rdr[:], r0[:])
                              Rsb = tiny.tile([HD, 512], F32, tag="Rsb")
                              rdrap = rdr[:]
                              bcast = bass.AP(tensor=rdrap.tensor,
                                              offset=rdrap.offset,
                                              ap=[[0, HD]] + list(rdrap.ap)[1:])
                              nc.sync.dma_start(Rsb[:], bcast)
                              # normalize on the idle Pool engine; the very
                              # last group gates the y tail, so use fast DVE
                              mul_eng = (nc.vector if (p == NPAIR - 1 and
                                                       G == TG - 1)
                                         else nc.gpsimd)
                              mul_eng.tensor_mul(
                                  oT[64 * sigma:64 * (sigma + 1), p,
                                     512 * G:512 * (G + 1)],
                                  oU[0:HD, :], Rsb[:])

                  # ---- phase 3: output projection ----
                  for G in range(TG):
                      emit_y_group(G)


    nc.compile()
    return nc


_RUNNER = None


def _make_pjrt_runner(nc):
    """Wrap a compiled Bass program as an 8-core PJRT callable."""
    import jax
    import numpy as _np
    from jax.sharding import Mesh, PartitionSpec
    from jax.experimental.shard_map import shard_map
    from concourse import bass2jax, mybir
    from concourse.bass2jax import (_bass_exec_p, install_neuronx_cc_hook,
                                    partition_id_tensor)

    install_neuronx_cc_hook()

    partition_name = (nc.partition_id_tensor.name
                      if nc.partition_id_tensor else None)
    in_names, out_names, out_avals = [], [], []
    for alloc in nc.m.functions[0].allocations:
        if not isinstance(alloc, mybir.MemoryLocationSet):
            continue
        if not alloc.memorylocations:
            continue
        name = alloc.memorylocations[0].name
        if alloc.kind == "ExternalInput":
            if name != partition_name:
                in_names.append(name)
        elif alloc.kind == "ExternalOutput":
            out_names.append(name)
            out_avals.append(jax.core.ShapedArray(
                tuple(alloc.tensor_shape), mybir.dt.np(alloc.dtype)))
    n_params = len(in_names)
    n_outs = len(out_names)
    zero_shapes = [(a.shape, a.dtype) for a in out_avals]
    all_in_names = in_names + out_names
    if partition_name is not None:
        all_in_names = all_in_names + [partition_name]

    def _body(*args):
        operands = list(args)
        if partition_name is not None:
            operands.append(partition_id_tensor())
        outs = _bass_exec_p.bind(
            *operands,
            out_avals=tuple(out_avals),
            in_names=tuple(all_in_names),
            out_names=tuple(out_names),
            lowering_input_output_aliases=(),
            sim_require_finite=True,
            sim_require_nnan=True,
            nc=nc,
        )
        return tuple(outs)

    devices = jax.devices()[:NCORES]
    mesh = Mesh(np.asarray(devices), ("core",))
    sharded = jax.jit(
        shard_map(_body, mesh=mesh,
                  in_specs=(PartitionSpec("core"),) * (n_params + n_outs),
                  out_specs=(PartitionSpec("core"),) * n_outs,
                  check_rep=False),
        keep_unused=True,
    )

    def run(in_maps):
        concat_in = [
            _np.concatenate([_np.asarray(in_maps[c][n]) for c in range(NCORES)],
                            axis=0)
            for n in in_names
        ]
        concat_zeros = [
            _np.zeros((NCORES * s[0], *s[1:]), d) for (s, d) in zero_shapes
        ]
        out_arrs = sharded(*concat_in, *concat_zeros)
        return [
            {
                n: _np.asarray(out_arrs[i]).reshape(NCORES, *out_avals[i].shape)[c]
                for i, n in enumerate(out_names)
            }
            for c in range(NCORES)
        ]

    internals = dict(nc=nc, body=_body, mesh=mesh, in_names=in_names,
                     out_names=out_names, zero_shapes=zero_shapes,
                     n_params=n_params)
    return run, in_names, internals


def _get_runner():
    """Build the Bass program once and return a cached 8-core PJRT callable."""
    global _RUNNER, _INTERNALS
    if _RUNNER is not None:
        return _RUNNER
    run, in_names, internals = _make_pjrt_runner(_build_program())
    _INTERNALS = internals
    _RUNNER = (run, in_names)
    return _RUNNER


def _make_in_maps(x, wq, wk, wv, wo):
    x = np.asarray(x, np.float32)
    wq_s = np.asarray(wq, np.float32) * (1.0 / np.sqrt(HD))  # fold score scale
    wk = np.asarray(wk, np.float32)
    wv = np.asarray(wv, np.float32)
    wo = np.asarray(wo, np.float32)

    xt_b = [_round_f32r(x[b].T) for b in range(B)]
    in_maps = []
    for c in range(NCORES):
        b, g = c // GROUPS, c % GROUPS
        sl = slice(DH * g, DH * (g + 1))
        in_maps.append({
            "xt": xt_b[b],
            "wqt": _round_f32r(wq_s[sl, :].T),
            "wkt": _round_f32r(wk[sl, :].T),
            "wvt": _round_f32r(wv[sl, :].T),
            "wot": _round_f32r(wo[:, sl].T),
        })
    return in_maps


def kernel(x, wq, wk, wv, wo):
    run, _ = _get_runner()
    results = run(_make_in_maps(x, wq, wk, wv, wo))
    y = np.zeros((B, T, DIM), np.float32)
    for c in range(NCORES):
        y[c // GROUPS] += results[c]["y"]
    return y

